# revision 48
# baseline (speedup 1.0000x reference)
"""Trainium2 Bass kernel for CapsuleLayer dynamic routing.

Problem: inputs [64, 2048, 8] f32, W [32, 2048, 16, 8] f32
  inputs_hat[b,n,i,e] = sum_d inputs[b,i,d] * W[n,i,e,d]
  3 routing iterations (softmax over n, weighted sums over i, squash)
  -> outputs [64, 32, 16] f32

Strategy: data-parallel over batch across 8 cores (8 batches each, W
replicated).  Per core:
  Phase 1 (DMA-paced, ~85us): stream W (pre-cast bf16, p-major host
    layout -> contiguous 4KB partition lines per 4-chunk DMA group) AND
    the block-diagonal-x lhs chunks bprep (pre-built on host:
    bp_k[(i,d),(i',b)] = x[b,i,d] iff i==i'), so the PE feed depends
    only on DMA.  inputs_hat via bprep matmuls on the PE.  PSUM->SBUF
    bf16 evac split 3:1 ACT:DVE.  s_0 = sum_i ih accumulates on DVE in
    half-slab (4-chunk) adds issued as soon as their evacs land (short
    adds so the 6-deep PSUM pipeline never stalls), then one dmask
    matmul folds (i16, slots).
  Phase 2 (routing tail, on-chip, DVE-bound ~97% busy): ih[p=(i16,b8),
    f=(k,e,n)] so softmax-n, the e-contraction (b-update) and the
    i-contraction all coexist: every big DVE multiply hits the 2x bf16
    mode (innermost n, step 1; broadcasts on outer/middle dims).
    b_r = (sum_{r'<r} v_r') . ih, so no b accumulator is stored;
    iteration 2 multiplies by u = v0+v1 (one tiny add).  b-update via
    DVE mul + contiguous e-halving tree.  1/Z is folded into exp(b) on
    the Pool engine (cb = eb*rec) so the s-matmul lhs is the static
    dmask.  The block loop is software-pipelined: p2chain(b+1) runs on
    DVE while ACT computes exp(b), so DVE never waits on the exp; p3 is
    produced in kb-halves so the s-matmuls issue earlier and the
    end-of-iteration PE drain is short.  squash via Ln/Exp (one ACT
    table set, no sqrt table switch).  v replicated to 128 partitions
    via a PE broadcast matmul (rep-mask lhs) + ACT evac.

Known walls (measured): phase-2 DVE busy ~108us/iteration is the 2x
bf16 DVE roofline for p2/p3 multiplies + e-tree; Pool TT runs at 0.42
efficiency and shares the SBUF port with DVE, so offloading the big
multiplies/tree levels to Pool regresses (tested); matmul PSUM outputs
at non-zero free offset are rejected by this walrus build (no
1024-wide paired outputs).
"""

import numpy as np

B, I, DI = 64, 2048, 8
N, DO = 32, 16
CORES = 8
BL = B // CORES  # 8 batches per core
KC = 128         # i-chunks
ISUB = 16        # i per chunk
FNE = N * DO     # 512
KB = 16          # chunks per tail block
NBLK = KC // KB  # 8 tail blocks
WG = 4           # W chunks per DMA
EPS = 1e-7

_CACHE = {}


def _patch_tile_tail_barrier():
    """The walrus build in this container rejects >1 sync-wait on the Tile
    tail Drain.  Replace the multi-wait drain with one wait_ge per
    outstanding semaphore (SP executes them in order), then a bare drain."""
    import concourse.tile as tile

    if getattr(tile.TileContext, "_ant_split_drain_patch", False):
        return

    def _drain_and_barrier(self, tick_clock, wait_clock):
        gc = tick_clock.global_clock
        ticks = eval(repr(gc).replace("VectorClock(", "").rstrip(")"))
        for idx, sem in sorted(self.sems.allocated().items()):
            if idx < len(ticks) and ticks[idx] > 0:
                mult = 16 if idx >= 11 else 1
                self.nc.sync.wait_ge(sem, ticks[idx] * mult)
        self.nc.sync.drain()
        self.nc.all_engine_barrier()
        popped = self.nc._tile_sem_poison_stack.pop()
        assert popped is self._sem_poison
        self.nc.clear_and_free_semaphores(list(self.sems.allocated().values()))

    tile.TileContext._drain_and_barrier = _drain_and_barrier
    tile.TileContext._ant_split_drain_patch = True


def _split_multi_waits(bir_bytes):
    """This container's walrus build allows only one sync-wait per
    instruction.  Hoist extra semaphore waits onto preceding wait-only
    EventSemaphore instructions on the same engine (engines execute their
    stream in order, so semantics are preserved)."""
    import json

    d = json.loads(bir_bytes)
    ctr = 0
    for f in d["functions"]:
        for blk in f["blocks"]:
            out = []
            for ins in blk["instructions"]:
                waits = ins.get("sync_info", {}).get("on_wait", [])
                if len(waits) > 1:
                    for w in waits[:-1]:
                        ctr += 1
                        out.append({
                            "debug": ins.get("debug", 0),
                            "engine": ins["engine"],
                            "ins": [],
                            "name": f"antwaitsplit-{ctr}",
                            "opcode": "EventSemaphore",
                            "outs": [],
                            "sync_info": {"on_update": [], "on_wait": [w]},
                        })
                    ins["sync_info"]["on_wait"] = [waits[-1]]
                out.append(ins)
            blk["instructions"] = out
    return json.dumps(d).encode()


def _patch_compile_split_waits():
    from concourse import bass2jax, bass_utils

    if getattr(bass_utils, "_ant_split_waits_patch", False):
        return
    orig = bass_utils.compile_bir_kernel

    def patched(bir_json, tmpdir, neff_name="file.neff"):
        return orig(_split_multi_waits(bir_json), tmpdir, neff_name)

    bass_utils.compile_bir_kernel = patched
    bass_utils._ant_split_waits_patch = True
    if getattr(bass2jax, "compile_bir_kernel", None) is orig:
        bass2jax.compile_bir_kernel = patched


def _build_nc():
    import concourse.bass as bass
    import concourse.tile as tile
    from concourse import mybir

    _patch_tile_tail_barrier()
    _patch_compile_split_waits()

    f32 = mybir.dt.float32
    bf16 = mybir.dt.bfloat16
    AF = mybir.ActivationFunctionType
    OP = mybir.AluOpType
    AX = mybir.AxisListType

    nc = bass.Bass(target_bir_lowering=False)

    wprep = nc.dram_tensor("wprep", [128, KC, FNE], bf16, kind="ExternalInput")
    bprep = nc.dram_tensor("bprep", [128, KC, 128], bf16, kind="ExternalInput")
    dmask = nc.dram_tensor("dmask", [128, BL], bf16, kind="ExternalInput")
    rmask = nc.dram_tensor("rmask", [BL, 128], bf16, kind="ExternalInput")
    out_d = nc.dram_tensor("out", [BL, FNE], f32, kind="ExternalOutput")

    dma_engines = [nc.sync, nc.gpsimd]

    with tile.TileContext(nc) as tc:
        with (
            tc.tile_pool(name="big", bufs=1) as big,
            tc.tile_pool(name="spsum", bufs=1, space="PSUM") as spp,
            tc.tile_pool(name="small", bufs=1) as small,
            tc.tile_pool(name="consts", bufs=1) as consts,
        ):
            # persistent tensors; ih free dims = (k, e, n)
            ih = big.tile([128, KC, DO, N], bf16, name="ih")
            dm = consts.tile([128, BL], bf16, name="dm")
            rm = consts.tile([BL, 128], bf16, name="rm")
            epsb = consts.tile([BL, 1], f32, name="epsb")
            nc.vector.memset(epsb[:], EPS)

            # ---------------- Phase 1: W stream ----------------
            # s0 = sum_i ih accumulates on DVE (8 k-slots, bf16) chasing the
            # evac stream; one final dmask matmul folds (i16, slots) in f32.
            s0 = spp.tile([BL, FNE], f32, name="s0")
            SLOTS = 8
            with (
                tc.tile_pool(name="wstream", bufs=8) as wpool,
                tc.tile_pool(name="bstream", bufs=8) as bpool,
                tc.tile_pool(name="mmpsum", bufs=6, space="PSUM") as mmp,
                tc.tile_pool(name="s0pool", bufs=1) as s0pool,
            ):
                acc = s0pool.tile([128, SLOTS, DO, N], bf16, name="s0acc")
                for g in range(KC // WG):
                    weng = dma_engines[(g + 1) % 2]
                    beng = dma_engines[g % 2]
                    wc = wpool.tile([128, WG, FNE], bf16, name="wc")
                    weng.dma_start(wc[:], wprep[:, g * WG:(g + 1) * WG, :])
                    bpc = bpool.tile([128, WG, 128], bf16, name="bpc")
                    beng.dma_start(bpc[:], bprep[:, g * WG:(g + 1) * WG, :])
                    if g == 0:
                        nc.sync.dma_start(dm[:], dmask[:])
                        nc.sync.dma_start(rm[:], rmask[:])
                    for j in range(WG):
                        k = g * WG + j
                        ps = mmp.tile([128, FNE], f32, name="ps")
                        nc.tensor.matmul(ps[:], bpc[:, j, :], wc[:, j, :],
                                         start=True, stop=True,
                                         skip_group_check=True)
                        dst = ih[:, k, :, :].rearrange("p e n -> p (e n)")
                        if k % 4 == 3:
                            nc.vector.tensor_copy(dst, ps[:])
                        else:
                            nc.scalar.copy(dst, ps[:])
                    # k-slot accumulation in half-slabs of 4 chunks, issued as
                    # soon as those 4 chunks are evacuated (keeps each DVE add
                    # short so the PSUM pipeline never stalls behind it)
                    kk = (g + 1) * WG
                    t, h0 = (kk - 4) // SLOTS, (kk - 4) % SLOTS
                    sl = ih[:, kk - 4:kk, :, :]
                    al = acc[:, h0:h0 + 4, :, :]
                    if t == 0:
                        nc.vector.tensor_copy(al, sl)
                    else:
                        nc.vector.tensor_add(al, al, sl)
                # reduce slots and (i16, b8) partitions via 8 accumulating
                # dmask matmuls (PE is idle here; skips the DVE fold tree)
                for j in range(SLOTS):
                    nc.tensor.matmul(
                        s0[:], dm[:],
                        acc[:, j, :, :].rearrange("p e n -> p (e n)"),
                        start=(j == 0), stop=(j == SLOTS - 1),
                        skip_group_check=True)

            # ---------------- squash helper (cols are (e, n)) ---------
            def squash(s_psum, r):
                scale0 = (1.0 / N) if r == 0 else 1.0
                sqv = small.tile([BL, FNE], f32, name="sqv", tag="sqv")
                nc.scalar.activation(sqv[:], s_psum[:], AF.Square, scale=scale0)
                s2 = small.tile([BL, N], f32, name="s2", tag="s2")
                nc.vector.tensor_reduce(
                    s2[:], sqv[:].rearrange("b (e n) -> b n e", e=DO),
                    axis=AX.X, op=OP.add)
                l1 = small.tile([BL, N], f32, name="l1", tag="l1")
                nc.scalar.activation(l1[:], s2[:], AF.Ln, bias=1.0)
                l2 = small.tile([BL, N], f32, name="l2", tag="l2")
                nc.scalar.activation(l2[:], s2[:], AF.Ln, bias=epsb[:])
                tt = small.tile([BL, N], f32, name="tt", tag="tt")
                nc.vector.scalar_tensor_tensor(
                    tt[:], l2[:], -0.5, l1[:],
                    op0=OP.mult, op1=OP.subtract)
                sc = small.tile([BL, N], f32, name="sc", tag="sc")
                nc.scalar.activation(sc[:], tt[:], AF.Exp)
                # fold scale0 into sc so v_f reads s_psum directly (no
                # separate scaled copy of s)
                nc.vector.scalar_tensor_tensor(
                    sc[:], sc[:], scale0, s2[:], op0=OP.mult, op1=OP.mult)
                vdt = f32 if r == 2 else bf16
                v_f = small.tile([BL, DO, N], vdt, name="v_f", tag="v_f",
                                 bufs=2)
                nc.vector.tensor_tensor(
                    v_f[:], s_psum[:].rearrange("b (e n) -> b e n", e=DO),
                    sc[:, None, :].broadcast_to([BL, DO, N]), op=OP.mult)
                return v_f

            v_f = squash(s0, 0)

            # ---------------- routing iterations ----------------
            with (
                tc.tile_pool(name="blk", bufs=2) as blkpool,
                tc.tile_pool(name="vpsum", bufs=1, space="PSUM") as vpp,
                tc.tile_pool(name="spsum2", bufs=1, space="PSUM") as spp2,
            ):
              v_r1 = v_f
              for r in (1, 2):
                  # b_r = (sum_{r'<r} v_r') . ih, so iteration 2 multiplies
                  # by u = v0 + v1 directly -- no stored b accumulator.
                  if r == 2:
                      u_f = small.tile([BL, DO, N], bf16, name="u_f",
                                       tag="u_f")
                      nc.vector.tensor_add(u_f[:], v_r1[:], v_f[:])
                      vmul = u_f
                  else:
                      vmul = v_f
                  # replicate u to 128 partitions via PE broadcast matmul
                  vps = vpp.tile([128, FNE], f32, name="vps", tag="vps")
                  nc.tensor.matmul(vps[:], rm[:],
                                   vmul[:].rearrange("b e n -> b (e n)"),
                                   start=True, stop=True, skip_group_check=True)
                  vrep = small.tile([128, DO, N], bf16, name="vrep", tag="vrep")
                  nc.scalar.copy(vrep[:].rearrange("p e n -> p (e n)"), vps[:])

                  s_ps = spp2.tile([BL, FNE], f32, name="s_ps", tag="s_ps")
                  BLKS = [18] * 7 + [2]
                  OFFS = [sum(BLKS[:i]) for i in range(len(BLKS))]
                  KBM = max(BLKS)

                  def p2chain(blk):
                      # b-update for blk: p2 = ih * v with e OUTER in the
                      # free layout (contiguous tree levels).
                      k0, kb = OFFS[blk], BLKS[blk]
                      ihb_e = ih[:, k0:k0 + kb, :, :].rearrange(
                          "p kb e n -> p e kb n")
                      p2f = blkpool.tile([128, DO, KBM, N], bf16, name="p2",
                                         tag="pp", bufs=3)
                      p2 = p2f[:, :, 0:kb, :]
                      vb = vrep[:, :, None, :].broadcast_to([128, DO, kb, N])
                      nc.vector.tensor_tensor(p2, ihb_e, vb, op=OP.mult)
                      h = DO
                      while h > 2:
                          h //= 2
                          nc.vector.tensor_add(
                              p2[:, 0:h], p2[:, 0:h], p2[:, h:2 * h])
                      bscf = blkpool.tile([128, KBM, N], bf16, name="bsc",
                                          tag="bsc")
                      bslc = bscf[:, 0:kb, :]
                      nc.vector.tensor_add(bslc, p2[:, 0, :, :],
                                           p2[:, 1, :, :])
                      return bslc

                  def softmax_p3(blk, eb):
                      # DVE softmax tail + p3 + PE matmuls for a block whose
                      # exp(b) has had a block's worth of time to finish.
                      k0, kb = OFFS[blk], BLKS[blk]
                      ihb = ih[:, k0:k0 + kb, :, :]  # [p,kb,e,n]
                      # softmax denominator chain on the (idle) Pool engine,
                      # hidden under the next block's p2chain on DVE
                      nsf = blkpool.tile([128, KBM], f32, name="ns", tag="ns")
                      ns = nsf[:, 0:kb]
                      nc.vector.tensor_reduce(ns, eb, axis=AX.X, op=OP.add)
                      recf = blkpool.tile([128, KBM], f32, name="rec",
                                          tag="rec")
                      rec = recf[:, 0:kb]
                      nc.vector.reciprocal(rec, ns)
                      # fold 1/Z into eb: cb = eb * rec (so the s-matmul lhs
                      # is the static dmask -> single stationary operand)
                      cbf = blkpool.tile([128, KBM, N], bf16, name="cb",
                                         tag="cb")
                      cb = cbf[:, 0:kb, :]
                      nc.gpsimd.tensor_tensor(
                          cb, eb, rec[:, :, None].broadcast_to([128, kb, N]),
                          op=OP.mult)
                      # s partial: p3 = ih * (exp(b)/Z), PE reduces over i.
                      # Produced in kb-halves so the matmuls issue earlier
                      # (spreads PE work, shrinks end-of-iteration drain).
                      p3f = blkpool.tile([128, KBM, DO, N], bf16, name="p3",
                                         tag="pp", bufs=3)
                      hs = [(0, (kb + 1) // 2), ((kb + 1) // 2, kb)]
                      for (h0, h1) in hs:
                          if h0 == h1:
                              continue
                          nc.vector.tensor_tensor(
                              p3f[:, h0:h1, :, :], ihb[:, h0:h1, :, :],
                              cb[:, h0:h1, None, :].broadcast_to(
                                  [128, h1 - h0, DO, N]), op=OP.mult)
                          for kk in range(h0, h1):
                              k = k0 + kk
                              nc.tensor.matmul(
                                  s_ps[:], dm[:],
                                  p3f[:, kk, :, :].rearrange(
                                      "p e n -> p (e n)"),
                                  start=(k == 0), stop=(k == KC - 1),
                                  skip_group_check=True)

                  # software-pipelined: p2chain(b+1) runs on DVE while ACT
                  # computes exp(b), so DVE never waits on the exp.
                  pend = None
                  for blk in range(len(BLKS)):
                      bslc = p2chain(blk)
                      ebf = blkpool.tile([128, KBM, N], bf16, name="eb",
                                         tag="eb")
                      eb = ebf[:, 0:BLKS[blk], :]
                      nc.scalar.activation(eb, bslc, AF.Exp)
                      if pend is not None:
                          softmax_p3(*pend)
                      pend = (blk, eb)
                  softmax_p3(*pend)

                  v_f = squash(s_ps, r)

            nc.sync.dma_start(out_d[:], v_f[:].rearrange("b e n -> b (e n)"))

    return nc


def _host_prep(inputs, W):
    import ml_dtypes
    bf = ml_dtypes.bfloat16

    # W_prep [128, 128, 512]: [(i16,d8), k, (e,n)]  (p-major: contiguous
    # 4KB partition lines per 4-chunk DMA group)
    wt = np.transpose(W, (1, 3, 2, 0))  # [i, d, e, n]
    wflat = wt.reshape(KC, ISUB * DI, DO * N)
    wprep = np.ascontiguousarray(np.transpose(wflat, (1, 0, 2))).astype(bf)

    # delta mask [128=(i16,b8), 8]
    dmask = np.tile(np.eye(BL, dtype=np.float32), (ISUB, 1)).astype(bf)
    # v replication mask: out row (g16, b8) <- v row b
    rmask = np.tile(np.eye(BL, dtype=np.float32), (1, ISUB)).astype(bf)

    in_maps = []
    eye16 = np.eye(ISUB, dtype=np.float32)
    for c in range(CORES):
        ic = inputs[c * BL:(c + 1) * BL]  # [8, 2048, 8]
        # bprep [(i16,d8), k, (i'16,b8)] = x[b, k*16+i, d] iff i == i'
        xt = ic.reshape(BL, KC, ISUB, DI).transpose(2, 3, 1, 0)  # [i,d,k,b]
        bp = np.einsum('ij,idkb->idkjb', eye16, xt)  # [i,d,k,i',b]
        bprep = np.ascontiguousarray(
            bp.reshape(ISUB * DI, KC, ISUB * BL)).astype(bf)
        in_maps.append({"wprep": wprep, "bprep": bprep,
                        "dmask": dmask, "rmask": rmask})
    return in_maps


def kernel(inputs, W):
    from concourse.bass_utils import run_bass_kernel_spmd

    inputs = np.asarray(inputs, dtype=np.float32)
    W = np.asarray(W, dtype=np.float32)

    if "nc" not in _CACHE:
        _CACHE["nc"] = _build_nc()
    nc = _CACHE["nc"]

    in_maps = _host_prep(inputs, W)
    res = run_bass_kernel_spmd(nc, in_maps, core_ids=list(range(CORES)))
    outs = [res.results[c]["out"].reshape(BL, DO, N).transpose(0, 2, 1)
            for c in range(CORES)]
    return np.concatenate(outs, axis=0).astype(np.float32)

